# revision 4
# baseline (speedup 1.0000x reference)
"""GRU cell on 8 Trainium2 NeuronCores.

Reference computation (B=65536, D=256):
    z = sigmoid(x@Wz + h@Uz + bz)
    r = sigmoid(x@Wr + h@Ur + br)
    h_hat = tanh(x@Wh + (r*h)@Uh + bh)
    h_t = z*h + (1-z)*h_hat  ; returns (h_t, h_t)

Strategy: data-parallel over the batch dim (8 shards of 8192 rows), all
fp16 on chip (rel_l2 ~1.1e-3 vs the f32 reference; gate is 2e-2).  The
matmul stream runs at the fp16 PE issue floor (24 matmul passes over
8192 batch cols = 196608 PE cycles = 81.9us at 2.4GHz), so everything
else is about the head (framework preamble 7.2us + first-DMA landing)
and the tail (last ACT/DVE chain + final store receipt + postamble).
Key structure vs the 104.4us version:
  * head DMAs split fine (12x65KB weight pieces + 256-col x/h tiles)
    and spread over sync/scalar/vector/gpsimd trigger queues in
    need-order: HWDGE rings are FIFO per issuing engine, so the
    critical first pieces are never queued behind bulk bytes.  8 HWDGE
    + 8 SWDGE completion-sem lanes bound the immediate-trigger budget.
  * the first two work items are 256 cols wide so the first real
    matmuls need only 65KB tiles; the last 512 cols are split
    256+128+128 so the serial ACT+DVE+store tail after the final
    matmul is short.
  * both activation tables are force-loaded early (dummy sigmoid+tanh
    on the warmup tile) - otherwise the tanh table load (1.28us) sits
    behind the scalar DMA triggers and stalls the first r-sigmoid,
    which showed up as candidate-matmul stalls in the trace.
  * warmup matmuls (N=256, cold 213ns each) hold the PE busy from
    ~7.5us so the HAM clock gate lifts 1.2->2.4GHz right as the first
    input tiles land (~10us); sized to end at data-arrival.
  * tail pieces use the short combine: u=sigmoid(-a_z) on ACT (bias
    -bz), m1=z*h early on DVE; after tanh only v=u*hh, o=v+m1 remain.
    Tail stores go 2-way on the idle sync/scalar HWDGE queues.
  * r-gate of item i+1 is computed one iteration early so its sigmoid
    + r*h never gate the candidate matmuls.
"""

import os
import sys

for _p in ("/opt/trn_rl_repo", "/root/.axon_site/_ro/trn_rl_repo"):
    if os.path.isdir(_p) and _p not in sys.path:
        sys.path.append(_p)

import numpy as np

B = 65536
D = 256
N_CORES = 8
S = B // N_CORES  # batch rows per core

# Input-tile load plan.
# Narrow per-block tiles for the first 512 cols (fast head fill), packed
# 4-block tiles for the bulk (DMA efficiency: 1KB+ per-partition lines).
NARROW = [(0, 256), (256, 256)]
PACKED = [(512, 512), (1024, 512), (1536, 512)] + [
    (2048 + 1024 * i, 1024) for i in range(6)
]
_BLOCKS = ("x0", "x1", "h0", "h1")
# matrix order inside the packed weight tensor
_WORDER = ("Wr", "Ur", "Wz", "Uz", "Wh", "Uh")

# Work items: (dram col start, width, psum col offset).  First two are
# narrow (head fill), last three shrink so the post-stream tail is short.
WI = (
    [(0, 256, 0), (256, 256, 256)]
    + [(512 + 512 * i, 512, 0) for i in range(14)]
    + [(7680, 256, 0), (7936, 128, 256), (8064, 128, 384)]
)


def build_nc(s=S, mm_dtype_name=None):
    """Build + compile the per-core Bass program for a shard of s rows."""
    import concourse.bass as bass
    import concourse.mybir as mybir
    import concourse.tile as tile
    from concourse import bacc

    f32 = mybir.dt.float32
    if mm_dtype_name is None:
        mm_dtype_name = os.environ.get("GRU_MM_DTYPE", "float16")
    f16 = getattr(mybir.dt, mm_dtype_name)
    AF = mybir.ActivationFunctionType
    n_warm = int(os.environ.get("GRU_WARMUP", "12"))

    nc = bacc.Bacc("TRN2", target_bir_lowering=False)
    xh = nc.dram_tensor("xh", [128, 4, s], f16, kind="ExternalInput")
    wcat = nc.dram_tensor("wcat", [D, 6 * D], f16, kind="ExternalInput")
    bcat = nc.dram_tensor("bcat", [128, 8], f32, kind="ExternalInput")
    outT = nc.dram_tensor("outT", [D, s], f16, kind="ExternalOutput")

    nwi = len(WI)

    with tile.TileContext(nc) as tc:
        with (
            tc.tile_pool(name="const", bufs=1) as cpool,
            tc.tile_pool(name="work", bufs=2) as wpool,
            tc.tile_pool(name="outb", bufs=4) as opool,
            tc.tile_pool(name="psum", bufs=1, space=bass.MemorySpace.PSUM) as ppool,
        ):
            # ---- PE warmup -------------------------------------------------
            # The HAM clock gate needs ~3.4us of sustained PE activity to
            # lift the engine from 1.2 to 2.4 GHz; the PE is idle during the
            # head DMAs anyway, so burn that window on dummy matmuls (cold
            # N=256 MM = 213ns each), sized to end when the input tiles land.
            zt0 = cpool.tile([128, 256], f16, tag="warm", name="warm")
            nc.gpsimd.memset(zt0[:], 0)
            pw = ppool.tile([128, 256], f32, tag="pwarm", name="pwarm")
            for _ in range(n_warm):
                nc.tensor.matmul(pw[:], zt0[:, 0:128], zt0[:], start=True, stop=True)
            wsink = cpool.tile([128, 256], f32, tag="wsink", name="wsink")
            nc.vector.tensor_copy(wsink[:], pw[:])

            # ---- head DMA schedule ----------------------------------------
            # Only sync/scalar (HWDGE) + gpsimd (SWDGE) can trigger DMAs.
            # HWDGE rings are FIFO per issuing engine and there are 8 HWDGE
            # + 8 SWDGE completion-sem lanes; each trigger instruction also
            # occupies its queue ~0.6us.  So: critical pieces first on their
            # rings in need-order, per-gate weight chunks (131KB) to keep
            # the trigger count affordable.
            wchunk = {}  # (gate 0..2, k) -> [128, 512] = W|U for that gate
            for c in range(3):
                for k in range(2):
                    wchunk[(c, k)] = cpool.tile(
                        [128, 512], f16, tag=f"w{c}{k}", name=f"w{c}{k}"
                    )

            def wload(c, k, eng):
                eng.dma_start(
                    wchunk[(c, k)][:],
                    wcat[k * 128 : (k + 1) * 128, c * 512 : (c + 1) * 512],
                )

            inp = {}  # (block, ('n'|'p', idx)) -> AP [128, width]

            def load_narrow(blk, li, eng):
                bi = _BLOCKS.index(blk)
                start, width = NARROW[li]
                t = cpool.tile([128, width], f16, tag=f"i{blk}_{li}",
                               name=f"i{blk}_{li}")
                eng.dma_start(t[:], xh[:, bi, start : start + width])
                inp[(blk, ("n", li))] = t[:]

            # sync: j=0 x tiles, then k=0 weight chunks in gate order
            load_narrow("x0", 0, nc.sync)
            load_narrow("x1", 0, nc.sync)
            wload(0, 0, nc.sync)
            wload(1, 0, nc.sync)
            wload(2, 0, nc.sync)
            # scalar: k=1 weight chunks in gate order
            wload(0, 1, nc.scalar)
            wload(1, 1, nc.scalar)
            wload(2, 1, nc.scalar)
            # gpsimd (SWDGE, own 8 lanes): j=0 h tiles, bias, j=1 tiles
            load_narrow("h0", 0, nc.gpsimd)
            load_narrow("h1", 0, nc.gpsimd)
            b_sb = cpool.tile([128, 8], f32, tag="bcat")
            nc.gpsimd.dma_start(b_sb[:], bcat[:, :])
            load_narrow("x0", 1, nc.gpsimd)
            load_narrow("x1", 1, nc.gpsimd)
            load_narrow("h0", 1, nc.gpsimd)
            load_narrow("h1", 1, nc.gpsimd)
            # force both ACT tables (sigmoid + tanh) to load now: lazily
            # the tanh load would sit mid-queue and stall the first sigmoid
            dume = cpool.tile([128, 1], f16, tag="dume", name="dume")
            nc.scalar.activation(dume[:], zt0[:, 0:1], AF.Sigmoid)
            nc.scalar.activation(dume[:], zt0[:, 0:1], AF.Tanh)
            # bulk input stream on sync's rotation-paced lanes
            for li, (start, width) in enumerate(PACKED):
                t = cpool.tile([128, 4, width], f16, tag=f"ixh_{li}",
                               name=f"ixh_{li}")
                nc.sync.dma_start(t[:], xh[:, :, start : start + width])
                for bi, blk in enumerate(_BLOCKS):
                    inp[(blk, ("p", li))] = t[:, bi, :]

            def wap(i, k, g):
                """Weight AP [128,128]: matrix i (order _WORDER),
                contraction half k, output-feature half g."""
                off = (i % 2) * 256 + g * 128
                return wchunk[(i // 2, k)][:, off : off + 128]

            def inp_ap(blk, c0, w):
                for li, (start, width) in enumerate(NARROW):
                    if start <= c0 and c0 + w <= start + width:
                        return inp[(blk, ("n", li))][:, c0 - start : c0 - start + w]
                for li, (start, width) in enumerate(PACKED):
                    if start <= c0 and c0 + w <= start + width:
                        return inp[(blk, ("p", li))][:, c0 - start : c0 - start + w]
                raise ValueError((blk, c0, w))

            def operands(i):
                c0, w, _ = WI[i]
                xs = [inp_ap(f"x{k}", c0, w) for k in range(2)]
                hs = [inp_ap(f"h{k}", c0, w) for k in range(2)]
                return xs, hs

            def gate_pair(tagbase, wi, ui, xs, rhs_u, po, w):
                """Both g-halves of one gate.  W (x-side) matmuls of both
                halves run before the U matmuls: x tiles arrive from HBM
                before h tiles, and for the candidate gate this gives the
                r*h producer extra slack.  k-major within each pass."""
                ps = []
                for g in range(2):
                    p = ppool.tile([128, 512], f32, tag=f"{tagbase}{g}",
                                   name=f"{tagbase}{g}")
                    ps.append(p)
                for k in range(2):
                    for g in range(2):
                        nc.tensor.matmul(ps[g][:, po : po + w], wap(wi, k, g),
                                         xs[k], start=(k == 0), stop=False)
                for k in range(2):
                    for g in range(2):
                        nc.tensor.matmul(ps[g][:, po : po + w], wap(ui, k, g),
                                         rhs_u[k], start=False, stop=(k == 1))
                return ps

            def r_gate(i):
                """reset gate -> r*h tiles for work item i."""
                c0, w, po = WI[i]
                xs, hs = operands(i)
                ps = gate_pair("pr", 0, 1, xs, hs, po, w)
                rh = []
                for g in range(2):
                    rt = wpool.tile([128, 512], f16, tag=f"r{g}", name=f"r{g}")
                    nc.scalar.activation(rt[:, 0:w], ps[g][:, po : po + w],
                                         AF.Sigmoid, bias=b_sb[:, g : g + 1])
                    t = wpool.tile([128, 512], f16, tag=f"rh{g}", name=f"rh{g}")
                    nc.vector.tensor_mul(t[:, 0:w], rt[:, 0:w], hs[g])
                    rh.append(t[:, 0:w])
                return rh

            # software pipeline: r-gate one work item ahead of z/candidate.
            rh_cur = r_gate(0)
            for i in range(nwi):
                c0, w, po = WI[i]
                xs, hs = operands(i)
                if i == 0:
                    rh_next = None
                elif i == 1:
                    rh_cur = r_gate(1)
                    rh_next = r_gate(2) if nwi > 2 else None
                else:
                    rh_next = r_gate(i + 1) if i + 1 < nwi else None

                tail = i >= nwi - 3
                pz = gate_pair("pz", 2, 3, xs, hs, po, w)
                zt, ut, m1 = [], [], []
                for g in range(2):
                    t = wpool.tile([128, 512], f16, tag=f"z{g}", name=f"z{g}")
                    nc.scalar.activation(t[:, 0:w], pz[g][:, po : po + w],
                                         AF.Sigmoid, bias=b_sb[:, 2 + g : 3 + g])
                    zt.append(t)
                    if tail:
                        # short-chain combine pieces, all off the critical
                        # path: u = 1-z = sigmoid(-a-bz), m1 = z*h
                        u = wpool.tile([128, 512], f16, tag=f"u{g}", name=f"u{g}")
                        nc.scalar.activation(u[:, 0:w], pz[g][:, po : po + w],
                                             AF.Sigmoid, bias=b_sb[:, 6 + g : 7 + g],
                                             scale=-1.0)
                        ut.append(u)
                        m = wpool.tile([128, 512], f16, tag=f"zh{g}", name=f"zh{g}")
                        nc.vector.tensor_mul(m[:, 0:w], t[:, 0:w], hs[g])
                        m1.append(m)

                ph = gate_pair("ph", 4, 5, xs, rh_cur, po, w)
                for g in range(2):
                    hh = wpool.tile([128, 512], f16, tag=f"hh{g}", name=f"hh{g}")
                    nc.scalar.activation(hh[:, 0:w], ph[g][:, po : po + w],
                                         AF.Tanh, bias=b_sb[:, 4 + g : 5 + g])
                    o = opool.tile([128, 512], f16, tag=f"o{g}", name=f"o{g}")
                    if tail:
                        # after tanh only two DVE links remain
                        v = wpool.tile([128, 512], f16, tag=f"v{g}", name=f"v{g}")
                        nc.vector.tensor_mul(v[:, 0:w], ut[g][:, 0:w], hh[:, 0:w])
                        nc.vector.tensor_add(o[:, 0:w], v[:, 0:w], m1[g][:, 0:w])
                    else:
                        d = wpool.tile([128, 512], f16, tag=f"d{g}", name=f"d{g}")
                        nc.vector.tensor_sub(d[:, 0:w], hs[g], hh[:, 0:w])
                        m = wpool.tile([128, 512], f16, tag=f"m{g}", name=f"m{g}")
                        nc.vector.tensor_mul(m[:, 0:w], zt[g][:, 0:w], d[:, 0:w])
                        nc.vector.tensor_add(o[:, 0:w], hh[:, 0:w], m[:, 0:w])
                    orow = outT[g * 128 : (g + 1) * 128, :]
                    # bulk stores ride gpsimd's SWDGE lanes so they never
                    # contend with the head/tail HWDGE traffic; the last
                    # three pieces' stores gate the postamble, so they go
                    # 2-way on the by-then-idle sync/scalar HWDGE queues.
                    if tail:
                        eng = nc.scalar if g == 1 else nc.sync
                    else:
                        eng = nc.gpsimd
                    eng.dma_start(orow[:, c0 : c0 + w], o[:, 0:w])
                rh_cur = rh_next

    nc.compile()
    return nc


_NC_CACHE = {}


def _get_nc():
    key = (S, os.environ.get("GRU_MM_DTYPE", "float16"),
           os.environ.get("GRU_WARMUP", "12"))
    if key not in _NC_CACHE:
        _NC_CACHE[key] = build_nc(S, key[1])
    return _NC_CACHE[key]


def _make_in_maps(inputs):
    f32 = np.float32
    dt16 = {"float16": np.float16}.get(
        os.environ.get("GRU_MM_DTYPE", "float16")
    )
    if dt16 is None:
        import ml_dtypes

        dt16 = ml_dtypes.bfloat16
    x = np.asarray(inputs["x"], f32)
    h = np.asarray(inputs["h_t_1"], f32)
    wcat = np.ascontiguousarray(
        np.concatenate(
            [np.asarray(inputs[n], f32) for n in ("Wr", "Ur", "Wz", "Uz", "Wh", "Uh")],
            axis=1,
        ).astype(dt16)
    )
    bz = np.asarray(inputs["bz"], f32)
    bcat = np.ascontiguousarray(
        np.concatenate(
            [
                np.asarray(inputs["br"], f32).reshape(2, 128).T,
                bz.reshape(2, 128).T,
                np.asarray(inputs["bh"], f32).reshape(2, 128).T,
                (-bz).reshape(2, 128).T,
            ],
            axis=1,
        )
    )
    consts = {"wcat": wcat, "bcat": bcat}
    in_maps = []
    for c in range(N_CORES):
        sl = slice(c * S, (c + 1) * S)
        xT = x[sl].T.astype(dt16)  # [256, S]
        hT = h[sl].T.astype(dt16)
        xhm = np.empty((128, 4, S), dt16)
        xhm[:, 0] = xT[0:128]
        xhm[:, 1] = xT[128:256]
        xhm[:, 2] = hT[0:128]
        xhm[:, 3] = hT[128:256]
        m = {"xh": np.ascontiguousarray(xhm)}
        m.update(consts)
        in_maps.append(m)
    return in_maps


def run(inputs, trace=False):
    """Run on hardware; returns (h_t ndarray, BassKernelResults)."""
    from concourse.bass_utils import run_bass_kernel_spmd

    nc = _get_nc()
    in_maps = _make_in_maps(inputs)
    res = run_bass_kernel_spmd(nc, in_maps, list(range(N_CORES)), trace=trace)
    out = np.empty((B, D), np.float32)
    for c in range(N_CORES):
        out[c * S : (c + 1) * S] = res.results[c]["outT"].T.astype(np.float32)
    return out, res


def kernel(**inputs):
    out, _ = run(inputs, trace=False)
    return (out, out)


# revision 8
# speedup vs baseline: 1.0152x; 1.0152x over previous
"""GRU cell on 8 Trainium2 NeuronCores.

Reference computation (B=65536, D=256):
    z = sigmoid(x@Wz + h@Uz + bz)
    r = sigmoid(x@Wr + h@Ur + br)
    h_hat = tanh(x@Wh + (r*h)@Uh + bh)
    h_t = z*h + (1-z)*h_hat  ; returns (h_t, h_t)

Strategy: data-parallel over the batch dim (8 shards of 8192 rows), all
fp16 on chip (rel_l2 ~1.1e-3 vs the f32 reference; gate is 2e-2).  The
matmul stream runs at the fp16 PE issue floor (24 matmul passes over
8192 batch cols = 196608 PE cycles = 81.9us at 2.4GHz), so everything
else is about the head (framework preamble 7.2us + first-DMA landing)
and the tail (last ACT/DVE chain + final store receipt + postamble).
Key structure vs the 104.4us version:
  * head DMAs split fine (12x65KB weight pieces + 256-col x/h tiles)
    and spread over sync/scalar/vector/gpsimd trigger queues in
    need-order: HWDGE rings are FIFO per issuing engine, so the
    critical first pieces are never queued behind bulk bytes.  8 HWDGE
    + 8 SWDGE completion-sem lanes bound the immediate-trigger budget.
  * the first two work items are 256 cols wide so the first real
    matmuls need only 65KB tiles; the last 512 cols are split
    256+128+128 so the serial ACT+DVE+store tail after the final
    matmul is short.
  * both activation tables are force-loaded early (dummy sigmoid+tanh
    on the warmup tile) - otherwise the tanh table load (1.28us) sits
    behind the scalar DMA triggers and stalls the first r-sigmoid,
    which showed up as candidate-matmul stalls in the trace.
  * warmup matmuls (N=256, cold 213ns each) hold the PE busy from
    ~7.5us so the HAM clock gate lifts 1.2->2.4GHz right as the first
    input tiles land (~10us); sized to end at data-arrival.
  * tail pieces use the short combine: u=sigmoid(-a_z) on ACT (bias
    -bz), m1=z*h early on DVE; after tanh only v=u*hh, o=v+m1 remain.
    Tail stores go 2-way on the idle sync/scalar HWDGE queues.
  * r-gate of item i+1 is computed one iteration early so its sigmoid
    + r*h never gate the candidate matmuls.
"""

import os
import sys

for _p in ("/opt/trn_rl_repo", "/root/.axon_site/_ro/trn_rl_repo"):
    if os.path.isdir(_p) and _p not in sys.path:
        sys.path.append(_p)

import numpy as np

B = 65536
D = 256
N_CORES = 8
S = B // N_CORES  # batch rows per core

# Input-tile load plan.
# Narrow per-block tiles for the first 512 cols (fast head fill), packed
# 4-block tiles for the bulk (DMA efficiency: 1KB+ per-partition lines).
NARROW = [(0, 256), (256, 256)]
PACKED = [(512, 512), (1024, 512), (1536, 512)] + [
    (2048 + 1024 * i, 1024) for i in range(6)
]
_BLOCKS = ("x0", "x1", "h0", "h1")
# matrix order inside the packed weight tensor
_WORDER = ("Wr", "Ur", "Wz", "Uz", "Wh", "Uh")

# Work items: (dram col start, width, psum col offset).  First two are
# narrow (head fill), last three shrink so the post-stream tail is short.
WI = (
    [(0, 256, 0), (256, 256, 256)]
    + [(512 + 512 * i, 512, 0) for i in range(14)]
    + [(7680, 256, 0), (7936, 256, 256)]
)


def build_nc(s=S, mm_dtype_name=None):
    """Build + compile the per-core Bass program for a shard of s rows."""
    import concourse.bass as bass
    import concourse.mybir as mybir
    import concourse.tile as tile
    from concourse import bacc

    f32 = mybir.dt.float32
    if mm_dtype_name is None:
        mm_dtype_name = os.environ.get("GRU_MM_DTYPE", "float16")
    f16 = getattr(mybir.dt, mm_dtype_name)
    AF = mybir.ActivationFunctionType
    n_warm = int(os.environ.get("GRU_WARMUP", "15"))

    nc = bacc.Bacc("TRN2", target_bir_lowering=False)
    xh = nc.dram_tensor("xh", [128, 4, s], f16, kind="ExternalInput")
    wcat = nc.dram_tensor("wcat", [D, 6 * D], f16, kind="ExternalInput")
    bcat = nc.dram_tensor("bcat", [128, 8], f32, kind="ExternalInput")
    outT = nc.dram_tensor("outT", [D, s], f16, kind="ExternalOutput")

    nwi = len(WI)

    with tile.TileContext(nc) as tc:
        with (
            tc.tile_pool(name="const", bufs=1) as cpool,
            tc.tile_pool(name="work", bufs=2) as wpool,
            tc.tile_pool(name="outb", bufs=4) as opool,
            tc.tile_pool(name="psum", bufs=1, space=bass.MemorySpace.PSUM) as ppool,
        ):
            # ---- PE warmup -------------------------------------------------
            # The HAM clock gate needs ~3.4us of sustained PE activity to
            # lift the engine from 1.2 to 2.4 GHz; the PE is idle during the
            # head DMAs anyway, so burn that window on dummy matmuls (cold
            # N=256 MM = 213ns each), sized to end when the input tiles land.
            zt0 = cpool.tile([128, 256], f16, tag="warm", name="warm")
            nc.gpsimd.memset(zt0[:], 0)
            pw = ppool.tile([128, 256], f32, tag="pwarm", name="pwarm")
            for _ in range(n_warm):
                nc.tensor.matmul(pw[:], zt0[:, 0:128], zt0[:], start=True, stop=True)
            wsink = cpool.tile([128, 256], f32, tag="wsink", name="wsink")
            nc.vector.tensor_copy(wsink[:], pw[:])

            # ---- head DMA schedule ----------------------------------------
            # Only sync/scalar (HWDGE) + gpsimd (SWDGE) can trigger DMAs.
            # HWDGE rings are FIFO per issuing engine and there are 8 HWDGE
            # + 8 SWDGE completion-sem lanes; each trigger instruction also
            # occupies its queue ~0.6us.  So: critical pieces first on their
            # rings in need-order, per-gate weight chunks (131KB) to keep
            # the trigger count affordable.
            wchunk = {}  # (gate 0..2, k) -> [128, 512] = W|U for that gate
            for c in range(3):
                for k in range(2):
                    wchunk[(c, k)] = cpool.tile(
                        [128, 512], f16, tag=f"w{c}{k}", name=f"w{c}{k}"
                    )

            def wload(c, k, eng):
                eng.dma_start(
                    wchunk[(c, k)][:],
                    wcat[k * 128 : (k + 1) * 128, c * 512 : (c + 1) * 512],
                )

            inp = {}  # (block, ('n'|'p', idx)) -> AP [128, width]

            def load_narrow(blk, li, eng):
                bi = _BLOCKS.index(blk)
                start, width = NARROW[li]
                t = cpool.tile([128, width], f16, tag=f"i{blk}_{li}",
                               name=f"i{blk}_{li}")
                eng.dma_start(t[:], xh[:, bi, start : start + width])
                inp[(blk, ("n", li))] = t[:]

            # sync: j=0 x tiles, then k=0 weight chunks in gate order
            load_narrow("x0", 0, nc.sync)
            load_narrow("x1", 0, nc.sync)
            wload(0, 0, nc.sync)
            wload(1, 0, nc.sync)
            wload(2, 0, nc.sync)
            # scalar: k=1 weight chunks in gate order
            wload(0, 1, nc.scalar)
            wload(1, 1, nc.scalar)
            wload(2, 1, nc.scalar)
            def load_packed(li, eng):
                start, width = PACKED[li]
                t = cpool.tile([128, 4, width], f16, tag=f"ixh_{li}",
                               name=f"ixh_{li}")
                eng.dma_start(t[:], xh[:, :, start : start + width])
                for bi, blk in enumerate(_BLOCKS):
                    inp[(blk, ("p", li))] = t[:, bi, :]

            # gpsimd (SWDGE, own 8 lanes): j=0 h tiles, bias, then the
            # first packed tile (so it doesn't steal HBM bandwidth from
            # the critical head pieces on the HWDGE rings), then j=1 tiles
            load_narrow("h0", 0, nc.gpsimd)
            load_narrow("h1", 0, nc.gpsimd)
            b_sb = cpool.tile([128, 8], f32, tag="bcat")
            nc.gpsimd.dma_start(b_sb[:], bcat[:, :])
            load_packed(0, nc.gpsimd)
            load_narrow("x0", 1, nc.gpsimd)
            load_narrow("x1", 1, nc.gpsimd)
            load_narrow("h0", 1, nc.gpsimd)
            load_narrow("h1", 1, nc.gpsimd)
            # force both ACT tables (sigmoid + tanh) to load now: lazily
            # the tanh load would sit mid-queue and stall the first sigmoid
            dume = cpool.tile([128, 1], f16, tag="dume", name="dume")
            nc.scalar.activation(dume[:], zt0[:, 0:1], AF.Sigmoid)
            nc.scalar.activation(dume[:], zt0[:, 0:1], AF.Tanh)
            # bulk input stream on sync's rotation-paced lanes
            for li in range(1, len(PACKED)):
                load_packed(li, nc.sync)

            def wap(i, k, g):
                """Weight AP [128,128]: matrix i (order _WORDER),
                contraction half k, output-feature half g."""
                off = (i % 2) * 256 + g * 128
                return wchunk[(i // 2, k)][:, off : off + 128]

            def inp_ap(blk, c0, w):
                for li, (start, width) in enumerate(NARROW):
                    if start <= c0 and c0 + w <= start + width:
                        return inp[(blk, ("n", li))][:, c0 - start : c0 - start + w]
                for li, (start, width) in enumerate(PACKED):
                    if start <= c0 and c0 + w <= start + width:
                        return inp[(blk, ("p", li))][:, c0 - start : c0 - start + w]
                raise ValueError((blk, c0, w))

            def operands(i):
                c0, w, _ = WI[i]
                xs = [inp_ap(f"x{k}", c0, w) for k in range(2)]
                hs = [inp_ap(f"h{k}", c0, w) for k in range(2)]
                return xs, hs

            def gate_pair(tagbase, wi, ui, xs, rhs_u, po, w):
                """Both g-halves of one gate.  W (x-side) matmuls of both
                halves run before the U matmuls: x tiles arrive from HBM
                before h tiles, and for the candidate gate this gives the
                r*h producer extra slack.  k-major within each pass."""
                ps = []
                for g in range(2):
                    p = ppool.tile([128, 512], f32, tag=f"{tagbase}{g}",
                                   name=f"{tagbase}{g}")
                    ps.append(p)
                for k in range(2):
                    for g in range(2):
                        nc.tensor.matmul(ps[g][:, po : po + w], wap(wi, k, g),
                                         xs[k], start=(k == 0), stop=False)
                for k in range(2):
                    for g in range(2):
                        nc.tensor.matmul(ps[g][:, po : po + w], wap(ui, k, g),
                                         rhs_u[k], start=False, stop=(k == 1))
                return ps

            def r_gate(i):
                """reset gate -> r*h tiles for work item i."""
                c0, w, po = WI[i]
                xs, hs = operands(i)
                ps = gate_pair("pr", 0, 1, xs, hs, po, w)
                rh = []
                for g in range(2):
                    rt = wpool.tile([128, 512], f16, tag=f"r{g}", name=f"r{g}")
                    nc.scalar.activation(rt[:, 0:w], ps[g][:, po : po + w],
                                         AF.Sigmoid, bias=b_sb[:, g : g + 1])
                    t = wpool.tile([128, 512], f16, tag=f"rh{g}", name=f"rh{g}")
                    nc.vector.tensor_mul(t[:, 0:w], rt[:, 0:w], hs[g])
                    rh.append(t[:, 0:w])
                return rh

            # software pipeline: r-gate one work item ahead of z/candidate.
            rh_cur = r_gate(0)
            for i in range(nwi):
                c0, w, po = WI[i]
                xs, hs = operands(i)
                if i == 0:
                    rh_next = None
                elif i == 1:
                    rh_cur = r_gate(1)
                    rh_next = r_gate(2) if nwi > 2 else None
                else:
                    rh_next = r_gate(i + 1) if i + 1 < nwi else None

                tail = i >= nwi - 2
                pz = gate_pair("pz", 2, 3, xs, hs, po, w)
                zt, ut, m1 = [], [], []
                for g in range(2):
                    t = wpool.tile([128, 512], f16, tag=f"z{g}", name=f"z{g}")
                    nc.scalar.activation(t[:, 0:w], pz[g][:, po : po + w],
                                         AF.Sigmoid, bias=b_sb[:, 2 + g : 3 + g])
                    zt.append(t)
                    if tail:
                        # precompute (on DVE, off the critical tail path)
                        # u = z-1 and m1 = z*h so only two DVE links
                        # remain after the final tanh:
                        #   o = z*h + (1-z)*hh = m1 - u*hh
                        u = wpool.tile([128, 512], f16, tag=f"u{g}", name=f"u{g}")
                        nc.vector.tensor_scalar_sub(u[:, 0:w], t[:, 0:w], 1.0)
                        ut.append(u)
                        m = wpool.tile([128, 512], f16, tag=f"zh{g}", name=f"zh{g}")
                        nc.vector.tensor_mul(m[:, 0:w], t[:, 0:w], hs[g])
                        m1.append(m)

                ph = gate_pair("ph", 4, 5, xs, rh_cur, po, w)
                for g in range(2):
                    hh = wpool.tile([128, 512], f16, tag=f"hh{g}", name=f"hh{g}")
                    nc.scalar.activation(hh[:, 0:w], ph[g][:, po : po + w],
                                         AF.Tanh, bias=b_sb[:, 4 + g : 5 + g])
                    o = opool.tile([128, 512], f16, tag=f"o{g}", name=f"o{g}")
                    if tail:
                        v = wpool.tile([128, 512], f16, tag=f"v{g}", name=f"v{g}")
                        nc.vector.tensor_mul(v[:, 0:w], ut[g][:, 0:w], hh[:, 0:w])
                        nc.vector.tensor_sub(o[:, 0:w], m1[g][:, 0:w], v[:, 0:w])
                    else:
                        d = wpool.tile([128, 512], f16, tag=f"d{g}", name=f"d{g}")
                        nc.vector.tensor_sub(d[:, 0:w], hs[g], hh[:, 0:w])
                        m = wpool.tile([128, 512], f16, tag=f"m{g}", name=f"m{g}")
                        nc.vector.tensor_mul(m[:, 0:w], zt[g][:, 0:w], d[:, 0:w])
                        nc.vector.tensor_add(o[:, 0:w], hh[:, 0:w], m[:, 0:w])
                    orow = outT[g * 128 : (g + 1) * 128, :]
                    # bulk stores ride gpsimd's SWDGE lanes so they never
                    # contend with the head/tail HWDGE traffic; the last
                    # three pieces' stores gate the postamble, so they go
                    # 2-way on the by-then-idle sync/scalar HWDGE queues.
                    if tail:
                        eng = nc.scalar if g == 1 else nc.sync
                    else:
                        eng = nc.gpsimd
                    eng.dma_start(orow[:, c0 : c0 + w], o[:, 0:w])
                rh_cur = rh_next

    nc.compile()
    return nc


_NC_CACHE = {}


def _get_nc():
    key = (S, os.environ.get("GRU_MM_DTYPE", "float16"),
           os.environ.get("GRU_WARMUP", "12"))
    if key not in _NC_CACHE:
        _NC_CACHE[key] = build_nc(S, key[1])
    return _NC_CACHE[key]


def _make_in_maps(inputs):
    f32 = np.float32
    dt16 = {"float16": np.float16}.get(
        os.environ.get("GRU_MM_DTYPE", "float16")
    )
    if dt16 is None:
        import ml_dtypes

        dt16 = ml_dtypes.bfloat16
    x = np.asarray(inputs["x"], f32)
    h = np.asarray(inputs["h_t_1"], f32)
    wcat = np.ascontiguousarray(
        np.concatenate(
            [np.asarray(inputs[n], f32) for n in ("Wr", "Ur", "Wz", "Uz", "Wh", "Uh")],
            axis=1,
        ).astype(dt16)
    )
    bz = np.asarray(inputs["bz"], f32)
    bcat = np.ascontiguousarray(
        np.concatenate(
            [
                np.asarray(inputs["br"], f32).reshape(2, 128).T,
                bz.reshape(2, 128).T,
                np.asarray(inputs["bh"], f32).reshape(2, 128).T,
                (-bz).reshape(2, 128).T,
            ],
            axis=1,
        )
    )
    consts = {"wcat": wcat, "bcat": bcat}
    in_maps = []
    for c in range(N_CORES):
        sl = slice(c * S, (c + 1) * S)
        xT = x[sl].T.astype(dt16)  # [256, S]
        hT = h[sl].T.astype(dt16)
        xhm = np.empty((128, 4, S), dt16)
        xhm[:, 0] = xT[0:128]
        xhm[:, 1] = xT[128:256]
        xhm[:, 2] = hT[0:128]
        xhm[:, 3] = hT[128:256]
        m = {"xh": np.ascontiguousarray(xhm)}
        m.update(consts)
        in_maps.append(m)
    return in_maps


def run(inputs, trace=False):
    """Run on hardware; returns (h_t ndarray, BassKernelResults)."""
    from concourse.bass_utils import run_bass_kernel_spmd

    nc = _get_nc()
    in_maps = _make_in_maps(inputs)
    res = run_bass_kernel_spmd(nc, in_maps, list(range(N_CORES)), trace=trace)
    out = np.empty((B, D), np.float32)
    for c in range(N_CORES):
        out[c * S : (c + 1) * S] = res.results[c]["outT"].T.astype(np.float32)
    return out, res


def kernel(**inputs):
    out, _ = run(inputs, trace=False)
    return (out, out)
